# Initial kernel scaffold
#
"""Cross-modal triplet loss (margin ranking on hardest pos/neg pairs) on 8 trn2 NeuronCores.

Strategy (per sharding hint): shard rows of modal1 across the 8 cores (512 rows
each); replicate modal2 and targets. Each core computes its 512x4096 slab of the
pairwise squared-distance matrix with a single fused f32r matmul per tile:

    psum[m, j] = dot(m1[m], m2[j]) - sq1[m]/2 - sq2[j]/2 - (BIG/2) * mask[m, j]

The sq terms and the same-identity mask (64 ids, one-hot over 64 extra
"augmented" K-features) ride along as 68 extra contraction rows, so one PSUM
accumulation group yields  -2*psum = dist_sq + BIG*mask.  Row-wise min gives the
hardest-negative distance exactly (masked entries pushed up by BIG); row-wise
max gives BIG + hardest-positive dist_sq. sqrt is applied only to the final
per-row reductions (sqrt is monotone). Per-core loss/precision partial sums are
returned and combined on the host (mean over all 4096 rows).

modal2 arrives row-major; the contraction needs features on partitions, so m2
tiles are transposed on-chip via PE transpose-mode matmuls (f32r, 1.5 cyc/row)
and evacuated PSUM->SBUF by the vector/scalar engines.
"""

import functools

import numpy as np

import concourse.bass as bass
import concourse.mybir as mybir
import concourse.tile as tile
from concourse import bacc
from concourse.bass_utils import run_bass_kernel_spmd

F32 = mybir.dt.float32
F32R = mybir.dt.float32r
BF16 = mybir.dt.bfloat16
I32 = mybir.dt.int32
OP = mybir.AluOpType
AF = mybir.ActivationFunctionType
AX = mybir.AxisListType.X

N, D, NIDS, P = 4096, 2048, 64, 128
NCORES = 8
SH = N // NCORES      # 512 rows of modal1 per core
MT = SH // P          # 4 m-tiles per core
KT = D // P           # 16 k-tiles
CHUNK = 512           # modal2 rows per chunk (one PSUM bank of fp32)
NJC = N // CHUNK      # 8 chunks
JTC = CHUNK // P      # 4 j-tiles per chunk
KAUG = 128            # one-hot mask (0:64), sq1 pair (64:66), sq2 pair (96:98)
BIG = 16384.0         # > max dist_sq (~5000); power of two (exact in fp22)
EPS = 1e-12


def _hi_lo(nc, pool, vec, p):
    """Split [p,1] fp32 col into (hi, lo) pair, hi exactly bf16-representable.

    The PE truncates f32r operands to ~fp22 (13 mantissa bits). hi has 8
    mantissa bits and lo carries the remainder, so hi+lo survives the
    truncation with ~2^-23 relative error instead of 2^-14.
    """
    hb = pool.tile([p, 1], BF16, tag="hilo_b")
    nc.vector.tensor_copy(hb[:], vec[:])
    hl = pool.tile([p, 2], F32, tag="hilo")
    nc.vector.tensor_copy(hl[:, 0:1], hb[:])
    nc.vector.tensor_sub(hl[:, 1:2], vec[:], hl[:, 0:1])
    return hl


def _build(margin: float) -> bass.Bass:
    nc = bacc.Bacc(num_swdge_queues=4)
    m1s = nc.dram_tensor("m1s", [SH, D], F32, kind="ExternalInput")
    m2 = nc.dram_tensor("m2", [N, D], F32, kind="ExternalInput")
    tgt = nc.dram_tensor("tgt", [1, N], F32, kind="ExternalInput")
    tgts = nc.dram_tensor("tgts", [1, SH], F32, kind="ExternalInput")
    iden_d = nc.dram_tensor("iden", [P, P], F32, kind="ExternalInput")
    iota_d = nc.dram_tensor("iota", [NIDS, 1], F32, kind="ExternalInput")
    out_d = nc.dram_tensor("out", [2 * MT, 1], F32, kind="ExternalOutput")

    with tile.TileContext(nc) as tc:
        with (
            tc.tile_pool(name="const", bufs=1) as const,
            tc.tile_pool(name="m1t", bufs=KT) as m1tp,
            tc.tile_pool(name="nat", bufs=8) as natp,
            tc.tile_pool(name="m1np", bufs=MT) as m1np,
            tc.tile_pool(name="scr", bufs=1) as scrp,
            tc.tile_pool(name="m2t", bufs=KT + 1) as m2tp,
            tc.tile_pool(name="aug", bufs=2) as augp,
            tc.tile_pool(name="small", bufs=8) as smallp,
            tc.tile_pool(name="stat", bufs=2 * MT + 8) as statp,
            tc.tile_pool(name="psT", bufs=3, space=bass.MemorySpace.PSUM) as psT,
            tc.tile_pool(name="psD", bufs=MT, space=bass.MemorySpace.PSUM) as psD,
            tc.tile_pool(name="psS", bufs=1, space=bass.MemorySpace.PSUM) as psS,
        ):
            # ---- constants ----
            iden = const.tile([P, P], F32)
            nc.sync.dma_start(iden[:], iden_d[:, :])
            idenB = const.tile([P, P], F32R)
            nc.vector.tensor_copy(idenB[:], iden[:])
            idenF = iden[:]

            iota_f = const.tile([NIDS, 1], F32)
            nc.sync.dma_start(iota_f[:], iota_d[:, :])

            ones_col = const.tile([P, 1], F32)
            nc.vector.memset(ones_col[:], 1.0)
            zsrc = const.tile([P, CHUNK], F32)
            nc.vector.memset(zsrc[:], 0.0)

            # ---- lhsT aug features: [KAUG, SH] ----
            # rows 0:64: -BIG/2*onehot1; rows 64,65: hi/lo of -sq1/2;
            # all remaining rows 1.0 (sq2 rows pass through; rest hit rhs zeros)
            laug = const.tile([KAUG, SH], F32R)
            nc.vector.tensor_copy(laug[:, :], zsrc[:, :])
            nc.vector.tensor_scalar(
                laug[96:128, :], zsrc[96:128, :], 1.0, None, OP.add
            )
            bc1 = const.tile([NIDS, SH], F32)
            nc.sync.dma_start(bc1[:], tgts[0:1, :].broadcast_to((NIDS, SH)))
            nc.vector.tensor_scalar(
                laug[0:NIDS, :], bc1[:], iota_f[:], -BIG / 2.0, OP.is_equal, OP.mult
            )

            # ---- m1 shard: natural load, sq1, transpose to [k, m] ----
            m1n = []
            for mt in range(MT):
                t = m1np.tile([P, D], F32, tag="m1n", name=f"m1n{mt}")
                nc.sync.dma_start(t[:], m1s[mt * P : (mt + 1) * P, :])
                m1n.append(t)
                scr = scrp.tile([P, D], F32, tag="scr")
                s1c = smallp.tile([P, 1], F32, tag="sqc")
                nc.scalar.activation(scr[:], t[:], AF.Square, accum_out=s1c[:])
                v = smallp.tile([P, 1], F32, tag="sqv")
                nc.vector.tensor_scalar(v[:], s1c[:], -0.5, None, OP.mult)
                hl = _hi_lo(nc, smallp, v, P)
                pS = psS.tile([2, P], F32, tag="psS")
                nc.tensor.transpose(pS[:], hl[:], idenF)
                nc.vector.tensor_copy(laug[64:66, mt * P : (mt + 1) * P], pS[:])

            m1T = []
            for kt in range(KT):
                pt = psT.tile([P, SH], F32, tag="psT")
                for mt in range(MT):
                    nc.tensor.transpose(
                        pt[:, mt * P : (mt + 1) * P],
                        m1n[mt][:, kt * P : (kt + 1) * P],
                        idenF,
                    )
                dst = m1tp.tile([P, SH], F32R, tag="m1t")
                nc.vector.tensor_copy(dst[:], pt[:])
                m1T.append(dst)

            # ---- running per-row min/max of psum over chunks ----
            minb = [statp.tile([P, NJC], F32, tag="stat", name=f"minb{i}") for i in range(MT)]
            maxb = [statp.tile([P, NJC], F32, tag="stat", name=f"maxb{i}") for i in range(MT)]

            # ---- main loop over modal2 chunks ----
            pending_red = []
            for jc in range(NJC):
                # rhs aug features [KAUG, CHUNK]:
                # rows 0:64: onehot2; rows 64,65: ones (sq1 pass-through);
                # rows 96,97: hi/lo of -sq2/2; all other rows zero
                raug = augp.tile([KAUG, CHUNK], F32R, tag="aug")
                nc.vector.tensor_copy(raug[:, :], zsrc[:, :])
                nc.vector.tensor_scalar(
                    raug[64:96, :], zsrc[64:96, :], 1.0, None, OP.add
                )
                bc2 = augp.tile([NIDS, CHUNK], F32, tag="bc")
                nc.sync.dma_start(
                    bc2[:],
                    tgt[0:1, jc * CHUNK : (jc + 1) * CHUNK].broadcast_to(
                        (NIDS, CHUNK)
                    ),
                )
                nc.vector.tensor_scalar(
                    raug[0:NIDS, :], bc2[:], iota_f[:], None, OP.is_equal
                )

                m2n = []
                for jt in range(JTC):
                    j0 = jc * JTC + jt
                    t = natp.tile([P, D], F32R, tag="nat")
                    nc.gpsimd.dma_start(t[:], m2[j0 * P : (j0 + 1) * P, :])
                    m2n.append(t)
                    scr = scrp.tile([P, D], F32, tag="scr")
                    s2c = smallp.tile([P, 1], F32, tag="sqc")
                    nc.scalar.activation(
                        scr[:], t[:].bitcast(F32), AF.Square, accum_out=s2c[:]
                    )
                    v = smallp.tile([P, 1], F32, tag="sqv")
                    nc.vector.tensor_scalar(v[:], s2c[:], -0.5, None, OP.mult)
                    hl = _hi_lo(nc, smallp, v, P)
                    pS = psS.tile([2, P], F32, tag="psS")
                    nc.tensor.transpose(pS[:], hl[:], idenF)
                    nc.vector.tensor_copy(
                        raug[96:98, jt * P : (jt + 1) * P], pS[:]
                    )

                m2T = []

                def mm(mt, kt, pdt):
                    nc.tensor.matmul(
                        pdt[:],
                        m1T[kt][:, mt * P : (mt + 1) * P],
                        m2T[kt][:],
                        start=(kt == 0),
                        stop=False,
                    )

                def mm_aug(mt, pdt):
                    nc.tensor.matmul(
                        pdt[:],
                        laug[:, mt * P : (mt + 1) * P],
                        raug[:],
                        start=False,
                        stop=True,
                    )

                def reduce(mt, pdt, jc_):
                    nc.vector.tensor_reduce(
                        minb[mt][:, jc_ : jc_ + 1], pdt[:], AX, OP.min
                    )
                    nc.vector.tensor_reduce(
                        maxb[mt][:, jc_ : jc_ + 1], pdt[:], AX, OP.max
                    )

                # mt=0 pass: transpose m2 k-tiles (PE) one step ahead of the
                # MMs; reduces of the previous chunk's psum banks are popped
                # here so their slots free up before this chunk's allocations.
                pd0 = psD.tile([P, CHUNK], F32, tag="psD")
                for kt in range(KT):
                    if pending_red:
                        pending_red.pop(0)()
                    pt = psT.tile([P, CHUNK], F32R, tag="psT")
                    for jt in range(JTC):
                        nc.tensor.transpose(
                            pt[:, jt * P : (jt + 1) * P],
                            m2n[jt][:, kt * P : (kt + 1) * P],
                            idenB[:],
                        )
                    dst = m2tp.tile([P, CHUNK], F32R, tag="m2t")
                    if kt % 2 == 1:
                        nc.scalar.copy(dst[:], pt[:].bitcast(F32))
                    else:
                        nc.vector.tensor_copy(dst[:], pt[:].bitcast(F32))
                    m2T.append(dst)
                    if kt >= 1:
                        mm(0, kt - 1, pd0)
                mm(0, KT - 1, pd0)
                mm_aug(0, pd0)
                pending_red.append(lambda pdt=pd0, jc_=jc: reduce(0, pdt, jc_))

                for mt in range(1, MT):
                    pdt = psD.tile([P, CHUNK], F32, tag="psD")
                    for kt in range(KT):
                        mm(mt, kt, pdt)
                    mm_aug(mt, pdt)
                    pending_red.append(
                        lambda mt_=mt, pdt_=pdt, jc_=jc: reduce(mt_, pdt_, jc_)
                    )

            for r in pending_red:
                r()

            # ---- finale: per-row ap/an, loss, precision; column sums ----
            pmin = statp.tile([P, MT], F32, tag="fin")
            pmax = statp.tile([P, MT], F32, tag="fin")
            for mt in range(MT):
                nc.vector.tensor_reduce(
                    pmin[:, mt : mt + 1], minb[mt][:], AX, OP.min
                )
                nc.vector.tensor_reduce(
                    pmax[:, mt : mt + 1], maxb[mt][:], AX, OP.max
                )
            # ap_sq = max(-2*pmin - BIG, EPS); an_sq = max(-2*pmax, EPS)
            apq = statp.tile([P, MT], F32, tag="fin")
            nc.vector.tensor_scalar(apq[:], pmin[:], -2.0, BIG, OP.mult, OP.subtract)
            apq2 = statp.tile([P, MT], F32, tag="fin")
            nc.vector.tensor_scalar(apq2[:], apq[:], EPS, None, OP.max)
            anq = statp.tile([P, MT], F32, tag="fin")
            nc.vector.tensor_scalar(anq[:], pmax[:], -2.0, EPS, OP.mult, OP.max)

            prec = statp.tile([P, MT], F32, tag="fin")
            nc.vector.tensor_tensor(prec[:], anq[:], apq2[:], OP.is_gt)

            ap = statp.tile([P, MT], F32, tag="fin")
            nc.scalar.activation(ap[:], apq2[:], AF.Sqrt)
            an = statp.tile([P, MT], F32, tag="fin")
            nc.scalar.activation(an[:], anq[:], AF.Sqrt)

            lp = statp.tile([P, 2 * MT], F32, tag="fin2")
            nc.vector.tensor_sub(lp[:, 0:MT], ap[:], an[:])
            nc.vector.tensor_scalar(
                lp[:, 0:MT], lp[:, 0:MT], margin, 0.0, OP.add, OP.max
            )
            nc.vector.tensor_copy(lp[:, MT : 2 * MT], prec[:])

            pf = psS.tile([2 * MT, 1], F32, tag="psS")
            nc.tensor.matmul(pf[:], lp[:], ones_col[:])
            osb = statp.tile([2 * MT, 1], F32, tag="fin")
            nc.vector.tensor_copy(osb[:], pf[:])
            nc.sync.dma_start(out_d[:, :], osb[:])

    nc.finalize()
    return nc


@functools.lru_cache(maxsize=4)
def _get_program(margin: float) -> bass.Bass:
    return _build(margin)


def _make_in_maps(m1, m2, tgt_f32):
    iden = np.eye(P, dtype=np.float32)
    iota = np.arange(NIDS, dtype=np.float32).reshape(NIDS, 1)
    maps = []
    for c in range(NCORES):
        maps.append(
            {
                "m1s": np.ascontiguousarray(m1[c * SH : (c + 1) * SH]),
                "m2": m2,
                "tgt": tgt_f32,
                "tgts": np.ascontiguousarray(tgt_f32[:, c * SH : (c + 1) * SH]),
                "iden": iden,
                "iota": iota,
            }
        )
    return maps


def run(modal1_inputs, modal2_inputs, targets, margin, trace=False):
    m1 = np.ascontiguousarray(np.asarray(modal1_inputs, dtype=np.float32))
    m2 = np.ascontiguousarray(np.asarray(modal2_inputs, dtype=np.float32))
    tgt_f32 = np.asarray(targets).astype(np.float32).reshape(1, N)
    nc = _get_program(float(margin))
    res = run_bass_kernel_spmd(
        nc, _make_in_maps(m1, m2, tgt_f32), list(range(NCORES)), trace=trace
    )
    loss_sum = 0.0
    prec_sum = 0.0
    for r in res.results:
        o = r["out"].reshape(-1)
        loss_sum += float(o[:MT].sum())
        prec_sum += float(o[MT:].sum())
    loss = np.float32(loss_sum / N)
    prec = np.float32(prec_sum / N)
    return (loss, prec), res


def kernel(modal1_inputs, modal2_inputs, targets, margin):
    (loss, prec), _ = run(modal1_inputs, modal2_inputs, targets, margin)
    return loss, prec



# revision 1
# speedup vs baseline: 5.0691x; 5.0691x over previous
"""Cross-modal triplet loss (margin ranking on hardest pos/neg pairs) on 8 trn2 NeuronCores.

Strategy (per sharding hint): shard rows of modal1 across the 8 cores (512 rows
each); replicate modal2 and targets. Each core computes its 512x4096 slab of the
pairwise squared-distance matrix with a single fused f32r matmul per tile:

    psum[m, j] = dot(m1[m], m2[j]) - sq1[m]/2 - sq2[j]/2 - (BIG/2) * mask[m, j]

The sq terms and the same-identity mask (64 ids, one-hot over 64 extra
"augmented" K-features) ride along as 68 extra contraction rows, so one PSUM
accumulation group yields  -2*psum = dist_sq + BIG*mask.  Row-wise min gives the
hardest-negative distance exactly (masked entries pushed up by BIG); row-wise
max gives BIG + hardest-positive dist_sq. sqrt is applied only to the final
per-row reductions (sqrt is monotone). Per-core loss/precision partial sums are
returned and combined on the host (mean over all 4096 rows).

modal2 arrives row-major; the contraction needs features on partitions, so m2
tiles are transposed on-chip via PE transpose-mode matmuls (f32r, 1.5 cyc/row)
and evacuated PSUM->SBUF by the vector/scalar engines.
"""

import functools

import numpy as np

import concourse.bass as bass
import concourse.mybir as mybir
import concourse.tile as tile
from concourse import bacc
from concourse.bass_utils import run_bass_kernel_spmd

F32 = mybir.dt.float32
F32R = mybir.dt.float32r
BF16 = mybir.dt.bfloat16
I32 = mybir.dt.int32
OP = mybir.AluOpType
AF = mybir.ActivationFunctionType
AX = mybir.AxisListType.X

N, D, NIDS, P = 4096, 2048, 64, 128
NCORES = 8
SH = N // NCORES      # 512 rows of modal1 per core
MT = SH // P          # 4 m-tiles per core
KT = D // P           # 16 k-tiles
CHUNK = 512           # modal2 rows per chunk (one PSUM bank of fp32)
NJC = N // CHUNK      # 8 chunks
JTC = CHUNK // P      # 4 j-tiles per chunk
KAUG = 128            # one-hot mask (0:64), sq1 pair (64:66), sq2 pair (96:98)
BIG = 16384.0         # > max dist_sq (~5000); power of two (exact in fp22)
EPS = 1e-12


def _hi_lo(nc, pool, vec, p):
    """Split [p,1] fp32 col into (hi, lo) pair, hi exactly bf16-representable.

    The PE truncates f32r operands to ~fp22 (13 mantissa bits). hi has 8
    mantissa bits and lo carries the remainder, so hi+lo survives the
    truncation with ~2^-23 relative error instead of 2^-14.
    """
    hb = pool.tile([p, 1], BF16, tag="hilo_b")
    nc.vector.tensor_copy(hb[:], vec[:])
    hl = pool.tile([p, 2], F32, tag="hilo")
    nc.vector.tensor_copy(hl[:, 0:1], hb[:])
    nc.vector.tensor_sub(hl[:, 1:2], vec[:], hl[:, 0:1])
    return hl


def _build(margin: float) -> bass.Bass:
    nc = bacc.Bacc(num_swdge_queues=4)
    m1s = nc.dram_tensor("m1s", [SH, D], F32, kind="ExternalInput")
    m2 = nc.dram_tensor("m2", [N, D], F32, kind="ExternalInput")
    tgt = nc.dram_tensor("tgt", [1, N], F32, kind="ExternalInput")
    tgts = nc.dram_tensor("tgts", [1, SH], F32, kind="ExternalInput")
    iden_d = nc.dram_tensor("iden", [P, P], F32, kind="ExternalInput")
    iota_d = nc.dram_tensor("iota", [NIDS, 1], F32, kind="ExternalInput")
    out_d = nc.dram_tensor("out", [2 * MT, 1], F32, kind="ExternalOutput")

    with tile.TileContext(nc) as tc:
        with (
            tc.tile_pool(name="const", bufs=1) as const,
            tc.tile_pool(name="m1t", bufs=KT) as m1tp,
            tc.tile_pool(name="nat", bufs=8) as natp,
            tc.tile_pool(name="m1np", bufs=MT) as m1np,
            tc.tile_pool(name="scr", bufs=1) as scrp,
            tc.tile_pool(name="m2t", bufs=KT + 1) as m2tp,
            tc.tile_pool(name="aug", bufs=2) as augp,
            tc.tile_pool(name="small", bufs=8) as smallp,
            tc.tile_pool(name="stat", bufs=2 * MT + 8) as statp,
            tc.tile_pool(name="psT", bufs=3, space=bass.MemorySpace.PSUM) as psT,
            tc.tile_pool(name="psD", bufs=MT, space=bass.MemorySpace.PSUM) as psD,
            tc.tile_pool(name="psS", bufs=1, space=bass.MemorySpace.PSUM) as psS,
        ):
            # ---- constants ----
            iden = const.tile([P, P], F32)
            nc.sync.dma_start(iden[:], iden_d[:, :])
            idenB = const.tile([P, P], F32R)
            nc.vector.tensor_copy(idenB[:], iden[:])
            idenF = iden[:]

            iota_f = const.tile([NIDS, 1], F32)
            nc.sync.dma_start(iota_f[:], iota_d[:, :])

            ones_col = const.tile([P, 1], F32)
            nc.vector.memset(ones_col[:], 1.0)
            zsrc = const.tile([P, CHUNK], F32)
            nc.vector.memset(zsrc[:], 0.0)

            # ---- lhsT aug features: [KAUG, SH] ----
            # rows 0:64: -BIG/2*onehot1; rows 64,65: hi/lo of -sq1/2;
            # all remaining rows 1.0 (sq2 rows pass through; rest hit rhs zeros)
            laug = const.tile([KAUG, SH], F32R)
            nc.vector.tensor_copy(laug[:, :], zsrc[:, :])
            nc.vector.tensor_scalar(
                laug[96:128, :], zsrc[96:128, :], 1.0, None, OP.add
            )
            bc1 = const.tile([NIDS, SH], F32)
            nc.sync.dma_start(bc1[:], tgts[0:1, :].broadcast_to((NIDS, SH)))
            nc.vector.tensor_scalar(
                laug[0:NIDS, :], bc1[:], iota_f[:], -BIG / 2.0, OP.is_equal, OP.mult
            )

            # ---- m1 shard: natural load, sq1, transpose to [k, m] ----
            m1n = []
            for mt in range(MT):
                t = m1np.tile([P, D], F32, tag="m1n", name=f"m1n{mt}")
                nc.sync.dma_start(t[:], m1s[mt * P : (mt + 1) * P, :])
                m1n.append(t)
                scr = scrp.tile([P, D], F32, tag="scr")
                s1c = smallp.tile([P, 1], F32, tag="sqc")
                nc.scalar.activation(scr[:], t[:], AF.Square, accum_out=s1c[:])
                v = smallp.tile([P, 1], F32, tag="sqv")
                nc.vector.tensor_scalar(v[:], s1c[:], -0.5, None, OP.mult)
                hl = _hi_lo(nc, smallp, v, P)
                pS = psS.tile([2, P], F32, tag="psS")
                nc.tensor.transpose(pS[:], hl[:], idenF)
                nc.vector.tensor_copy(laug[64:66, mt * P : (mt + 1) * P], pS[:])

            m1T = []
            for kt in range(KT):
                pt = psT.tile([P, SH], F32, tag="psT")
                for mt in range(MT):
                    nc.tensor.transpose(
                        pt[:, mt * P : (mt + 1) * P],
                        m1n[mt][:, kt * P : (kt + 1) * P],
                        idenF,
                    )
                dst = m1tp.tile([P, SH], F32R, tag="m1t")
                nc.vector.tensor_copy(dst[:], pt[:])
                m1T.append(dst)

            # ---- running per-row min/max of psum over chunks ----
            minb = [statp.tile([P, NJC], F32, tag="stat", name=f"minb{i}") for i in range(MT)]
            maxb = [statp.tile([P, NJC], F32, tag="stat", name=f"maxb{i}") for i in range(MT)]

            # ---- main loop over modal2 chunks ----
            pending_red = []
            for jc in range(NJC):
                # rhs aug features [KAUG, CHUNK]:
                # rows 0:64: onehot2; rows 64,65: ones (sq1 pass-through);
                # rows 96,97: hi/lo of -sq2/2; all other rows zero
                raug = augp.tile([KAUG, CHUNK], F32R, tag="aug")
                nc.vector.tensor_copy(raug[:, :], zsrc[:, :])
                nc.vector.tensor_scalar(
                    raug[64:96, :], zsrc[64:96, :], 1.0, None, OP.add
                )
                bc2 = augp.tile([NIDS, CHUNK], F32, tag="bc")
                nc.sync.dma_start(
                    bc2[:],
                    tgt[0:1, jc * CHUNK : (jc + 1) * CHUNK].broadcast_to(
                        (NIDS, CHUNK)
                    ),
                )
                nc.vector.tensor_scalar(
                    raug[0:NIDS, :], bc2[:], iota_f[:], None, OP.is_equal
                )

                m2n = []
                for jt in range(JTC):
                    j0 = jc * JTC + jt
                    t = natp.tile([P, D], F32R, tag="nat")
                    nc.gpsimd.dma_start(t[:], m2[j0 * P : (j0 + 1) * P, :])
                    m2n.append(t)
                    scr = scrp.tile([P, D], F32, tag="scr")
                    s2c = smallp.tile([P, 1], F32, tag="sqc")
                    nc.scalar.activation(
                        scr[:], t[:].bitcast(F32), AF.Square, accum_out=s2c[:]
                    )
                    v = smallp.tile([P, 1], F32, tag="sqv")
                    nc.vector.tensor_scalar(v[:], s2c[:], -0.5, None, OP.mult)
                    hl = _hi_lo(nc, smallp, v, P)
                    pS = psS.tile([2, P], F32, tag="psS")
                    nc.tensor.transpose(pS[:], hl[:], idenF)
                    nc.vector.tensor_copy(
                        raug[96:98, jt * P : (jt + 1) * P], pS[:]
                    )

                m2T = []

                def mm(mt, kt, pdt):
                    nc.tensor.matmul(
                        pdt[:],
                        m1T[kt][:, mt * P : (mt + 1) * P],
                        m2T[kt][:],
                        start=(kt == 0),
                        stop=False,
                    )

                def mm_aug(mt, pdt):
                    nc.tensor.matmul(
                        pdt[:],
                        laug[:, mt * P : (mt + 1) * P],
                        raug[:],
                        start=False,
                        stop=True,
                    )

                def reduce(mt, pdt, jc_):
                    nc.vector.tensor_reduce(
                        minb[mt][:, jc_ : jc_ + 1], pdt[:], AX, OP.min
                    )
                    nc.vector.tensor_reduce(
                        maxb[mt][:, jc_ : jc_ + 1], pdt[:], AX, OP.max
                    )

                # mt=0 pass: transpose m2 k-tiles (PE) one step ahead of the
                # MMs; reduces of the previous chunk's psum banks are popped
                # here so their slots free up before this chunk's allocations.
                pd0 = psD.tile([P, CHUNK], F32, tag="psD")
                for kt in range(KT):
                    if pending_red:
                        pending_red.pop(0)()
                    pt = psT.tile([P, CHUNK], F32R, tag="psT")
                    for jt in range(JTC):
                        nc.tensor.transpose(
                            pt[:, jt * P : (jt + 1) * P],
                            m2n[jt][:, kt * P : (kt + 1) * P],
                            idenB[:],
                        )
                    dst = m2tp.tile([P, CHUNK], F32R, tag="m2t")
                    if kt % 2 == 1:
                        nc.scalar.copy(dst[:], pt[:].bitcast(F32))
                    else:
                        nc.vector.tensor_copy(dst[:], pt[:].bitcast(F32))
                    m2T.append(dst)
                    if kt >= 1:
                        mm(0, kt - 1, pd0)
                mm(0, KT - 1, pd0)
                mm_aug(0, pd0)
                pending_red.append(lambda pdt=pd0, jc_=jc: reduce(0, pdt, jc_))

                for mt in range(1, MT):
                    pdt = psD.tile([P, CHUNK], F32, tag="psD")
                    for kt in range(KT):
                        mm(mt, kt, pdt)
                    mm_aug(mt, pdt)
                    pending_red.append(
                        lambda mt_=mt, pdt_=pdt, jc_=jc: reduce(mt_, pdt_, jc_)
                    )

            for r in pending_red:
                r()

            # ---- finale: per-row ap/an, loss, precision; column sums ----
            pmin = statp.tile([P, MT], F32, tag="fin")
            pmax = statp.tile([P, MT], F32, tag="fin")
            for mt in range(MT):
                nc.vector.tensor_reduce(
                    pmin[:, mt : mt + 1], minb[mt][:], AX, OP.min
                )
                nc.vector.tensor_reduce(
                    pmax[:, mt : mt + 1], maxb[mt][:], AX, OP.max
                )
            # ap_sq = max(-2*pmin - BIG, EPS); an_sq = max(-2*pmax, EPS)
            apq = statp.tile([P, MT], F32, tag="fin")
            nc.vector.tensor_scalar(apq[:], pmin[:], -2.0, BIG, OP.mult, OP.subtract)
            apq2 = statp.tile([P, MT], F32, tag="fin")
            nc.vector.tensor_scalar(apq2[:], apq[:], EPS, None, OP.max)
            anq = statp.tile([P, MT], F32, tag="fin")
            nc.vector.tensor_scalar(anq[:], pmax[:], -2.0, EPS, OP.mult, OP.max)

            prec = statp.tile([P, MT], F32, tag="fin")
            nc.vector.tensor_tensor(prec[:], anq[:], apq2[:], OP.is_gt)

            ap = statp.tile([P, MT], F32, tag="fin")
            nc.scalar.activation(ap[:], apq2[:], AF.Sqrt)
            an = statp.tile([P, MT], F32, tag="fin")
            nc.scalar.activation(an[:], anq[:], AF.Sqrt)

            lp = statp.tile([P, 2 * MT], F32, tag="fin2")
            nc.vector.tensor_sub(lp[:, 0:MT], ap[:], an[:])
            nc.vector.tensor_scalar(
                lp[:, 0:MT], lp[:, 0:MT], margin, 0.0, OP.add, OP.max
            )
            nc.vector.tensor_copy(lp[:, MT : 2 * MT], prec[:])

            pf = psS.tile([2 * MT, 1], F32, tag="psS")
            nc.tensor.matmul(pf[:], lp[:], ones_col[:])
            osb = statp.tile([2 * MT, 1], F32, tag="fin")
            nc.vector.tensor_copy(osb[:], pf[:])
            nc.sync.dma_start(out_d[:, :], osb[:])

    nc.finalize()
    return nc


@functools.lru_cache(maxsize=4)
def _get_program(margin: float) -> bass.Bass:
    return _build(margin)


def _make_in_maps(m1, m2, tgt_f32):
    iden = np.eye(P, dtype=np.float32)
    iota = np.arange(NIDS, dtype=np.float32).reshape(NIDS, 1)
    maps = []
    for c in range(NCORES):
        maps.append(
            {
                "m1s": np.ascontiguousarray(m1[c * SH : (c + 1) * SH]),
                "m2": m2,
                "tgt": tgt_f32,
                "tgts": np.ascontiguousarray(tgt_f32[:, c * SH : (c + 1) * SH]),
                "iden": iden,
                "iota": iota,
            }
        )
    return maps


def run(modal1_inputs, modal2_inputs, targets, margin, trace=False):
    m1 = np.ascontiguousarray(np.asarray(modal1_inputs, dtype=np.float32))
    m2 = np.ascontiguousarray(np.asarray(modal2_inputs, dtype=np.float32))
    tgt_f32 = np.asarray(targets).astype(np.float32).reshape(1, N)
    nc = _get_program(float(margin))
    res = run_bass_kernel_spmd(
        nc, _make_in_maps(m1, m2, tgt_f32), list(range(NCORES)), trace=trace
    )
    loss_sum = 0.0
    prec_sum = 0.0
    for r in res.results:
        o = r["out"].reshape(-1)
        loss_sum += float(o[:MT].sum())
        prec_sum += float(o[MT:].sum())
    loss = np.float32(loss_sum / N)
    prec = np.float32(prec_sum / N)
    return (loss, prec), res


def kernel(modal1_inputs, modal2_inputs, targets, margin):
    (loss, prec), _ = run(modal1_inputs, modal2_inputs, targets, margin)
    return loss, prec

